# revision 2
# baseline (speedup 1.0000x reference)
"""Trainium2 Bass kernel v3: host-transposed (feature-major) input layout.

Host preps x as [P=128, q=8, r=128] per core, so every device op reads and
writes contiguous row-runs (slot-major).  Math via rsqrt-of-factors:

  c_q = rsqrt(x_q^2+1), d_q = rsqrt(x_q^4+1)  (q=0,1)
  K = c2..c7 prod, R2 = (c0 c1)(d0 d1), out = C0 + C1*c1*K
      + (w*R2)*(C2 + C3*(x1^2*K)),  w = x0*x1

Slots of s[P, 38, 128] bf16:
  0 x0^2 | 1 x1^2 | 2 x0^4 | 3 x1^4 | 4..9 x2^2..x7^2
  12 c0 | 13 c1 | 14 d0 | 15 d1 | 16..21 c2..c7
  22 c0c1 | 23 d0d1 | 24 c2c3 | 25 c4c5 | 26 c6c7
  27 R2 | 28 t=c4c5c6c7 | 29 K | 30 w | 31 v0=c1K | 37 v1=x1^2K
  32 m2=wR2 | 34 f2 | 35 f5 | 36 f4
"""

import numpy as np

import concourse.bass as bass
from concourse import mybir
from concourse.bass_utils import run_bass_kernel_spmd

N_CORES = 8
BATCH = 131072
NQ = 8
B_LOCAL = BATCH // N_CORES  # 16384
P = 128
R = B_LOCAL // P            # 128 rows per partition
NS = 38

F32 = mybir.dt.float32
BF16 = mybir.dt.bfloat16
AF = mybir.ActivationFunctionType
ALU = mybir.AluOpType


def _act_raw(nc, se, out, in_, func, bias=0.0):
    """InstActivation bypassing bass's Rsqrt guard (HW-validated in v1)."""
    b = nc.const_aps.scalar_like(bias, in_)
    ins = [se.lower_ap(in_), se.lower_ap(b),
           mybir.ImmediateValue(dtype=mybir.dt.float32, value=1.0),
           mybir.ImmediateValue(dtype=mybir.dt.float32, value=0.0)]
    return se.add_instruction(mybir.InstActivation(
        name=nc.get_next_instruction_name(), func=func,
        ins=ins, outs=[se.lower_ap(out)]))


def _build_nc():
    nc = bass.Bass()
    x = nc.declare_dram_parameter("x", [P, NQ, R], F32, isOutput=False)
    co = nc.declare_dram_parameter("co", [4], F32, isOutput=False)
    y = nc.declare_dram_parameter("y", [B_LOCAL], F32, isOutput=True)

    yv = y.rearrange("(p r) -> p r", p=P)          # [128, 128]
    co_ap = co[:]
    co_bcast = bass.AP(tensor=co_ap.tensor, offset=co_ap.offset,
                       ap=[[0, P], [1, 4]])

    import contextlib
    with contextlib.ExitStack() as ctx:
        ct = ctx.enter_context(nc.sbuf_tensor("ct", [P, 4], F32))
        junk = ctx.enter_context(nc.sbuf_tensor("junk", [P, 2], BF16))
        xq = ctx.enter_context(nc.sbuf_tensor("xq", [P, NQ, R], F32))
        s = ctx.enter_context(nc.sbuf_tensor("s", [P, NS, R], BF16))
        ot = ctx.enter_context(nc.sbuf_tensor("ot", [P, R], F32))

        s_in1 = ctx.enter_context(nc.semaphore("s_in1"))
        s_in2 = ctx.enter_context(nc.semaphore("s_in2"))
        s_ct = ctx.enter_context(nc.semaphore("s_ct"))
        s_act = ctx.enter_context(nc.semaphore("s_act"))
        s_sq67 = ctx.enter_context(nc.semaphore("s_sq67"))
        s_rc1 = ctx.enter_context(nc.semaphore("s_rc1"))
        s_rc2 = ctx.enter_context(nc.semaphore("s_rc2"))
        s_out = ctx.enter_context(nc.semaphore("s_out"))
        block = ctx.enter_context(nc.Block())

        def sl(start, n, stride=1):
            """s slots {start, start+stride, ...} x all rows, rows innermost."""
            a = s[:, start:start + 1, :]
            return bass.AP(tensor=a.tensor, offset=a.offset,
                           ap=[[a.ap[0][0], P], [stride * R, n], [1, R]])

        @block.sync
        def _(sync):
            sync.dma_start(out=xq[:, 0:2, :], in_=x[:, 0:2, :]
                           ).then_inc(s_in1, 16)
            sync.dma_start(out=xq[:, 2:8, :], in_=x[:, 2:8, :]
                           ).then_inc(s_in2, 16)
            sync.dma_start(out=ct[:], in_=co_bcast).then_inc(s_ct, 16)
            sync.wait_ge(s_out, 1)
            sync.dma_start(out=yv[:, :], in_=ot[:]).then_inc(s_ct, 16)

        @block.scalar
        def _(scalar):
            _act_raw(nc, scalar, junk[:, 1:2], junk[:, 0:1], AF.Rsqrt)
            scalar.wait_ge(s_in1, 16)
            # sq01: x0^2,x1^2 -> slots 0,1
            scalar.activation(s[:, 0:2, :], xq[:, 0:2, :],
                              AF.Square).then_inc(s_act, 1)
            scalar.wait_ge(s_act, 1)
            # x^4 -> slots 2,3
            scalar.activation(s[:, 2:4, :], s[:, 0:2, :],
                              AF.Square).then_inc(s_act, 1)
            scalar.wait_ge(s_act, 2)
            # rc1: rsqrt(slots 0:4 + 1) -> slots 12..15 = [c0,c1,d0,d1]
            _act_raw(nc, scalar, s[:, 12:16, :], s[:, 0:4, :],
                     AF.Rsqrt, bias=1.0).then_inc(s_rc1, 1)
            # sq2345: x2..x5 -> slots 4..7
            scalar.wait_ge(s_in2, 16)
            scalar.activation(s[:, 4:8, :], xq[:, 2:6, :],
                              AF.Square).then_inc(s_act, 1)
            scalar.wait_ge(s_act, 3)
            scalar.wait_ge(s_sq67, 1)
            # rc2: rsqrt(slots 4:10 + 1) -> slots 16..21 = c2..c7
            _act_raw(nc, scalar, s[:, 16:22, :], s[:, 4:10, :],
                     AF.Rsqrt, bias=1.0).then_inc(s_rc2, 1)

        @block.vector
        def _(vector):
            vector.wait_ge(s_in2, 16)
            # sq67: x6,x7 -> slots 8,9
            vector.tensor_mul(s[:, 8:10, :], xq[:, 6:8, :],
                              xq[:, 6:8, :]).then_inc(s_sq67, 1)
            vector.wait_ge(s_in1, 16)
            # w = x0*x1 -> slot 30
            vector.tensor_mul(s[:, 30:31, :], xq[:, 0:1, :], xq[:, 1:2, :])
            vector.wait_ge(s_rc1, 1)
            # L1a: [c0c1, d0d1] -> 22,23
            vector.tensor_mul(sl(22, 2), sl(12, 2, 2), sl(13, 2, 2))
            # R2 = c0c1 * d0d1 -> 27
            vector.tensor_mul(sl(27, 1), sl(22, 1), sl(23, 1))
            # m2 = w * R2 -> 32
            vector.tensor_mul(sl(32, 1), sl(30, 1), sl(27, 1))
            vector.wait_ge(s_rc2, 1)
            # L1b: [c2c3, c4c5, c6c7] -> 24,25,26
            vector.tensor_mul(sl(24, 3), sl(16, 3, 2), sl(17, 3, 2))
            # t = c4c5 * c6c7 -> 28 ; K = c2c3 * t -> 29
            vector.tensor_mul(sl(28, 1), sl(25, 1), sl(26, 1))
            vector.tensor_mul(sl(29, 1), sl(24, 1), sl(28, 1))
            # v = [c1*K -> 31, x1^2*K -> 37]
            vector.tensor_mul(sl(31, 2, 6), sl(13, 2, -12), sl(29, 2, 0))
            # f2 = C3*v1 + C2 -> 34 ; f5 = C1*v0 + C0 -> 35
            vector.wait_ge(s_ct, 16)
            vector.tensor_scalar(sl(34, 1), sl(37, 1), ct[:, 3:4], ct[:, 2:3],
                                 ALU.mult, ALU.add)
            vector.tensor_scalar(sl(35, 1), sl(31, 1), ct[:, 1:2], ct[:, 0:1],
                                 ALU.mult, ALU.add)
            # f4 = m2 * f2 -> 36 ; out = f4 + f5
            vector.tensor_mul(sl(36, 1), sl(32, 1), sl(34, 1))
            vector.tensor_add(
                ot[:, :],
                s[:, 36:37, :].rearrange("p one r -> p (one r)"),
                s[:, 35:36, :].rearrange("p one r -> p (one r)")
            ).then_inc(s_out, 1)

    return nc


_NC = None


def _get_nc():
    global _NC
    if _NC is None:
        _NC = _build_nc()
    return _NC


def _host_coeffs(weights_re, weights_im):
    w = (np.asarray(weights_re, np.float64)
         + 1j * np.asarray(weights_im, np.float64)) * 0.5
    c, s = np.cos(w), np.sin(w)

    def rymat(i):
        return np.array([[c[i], -s[i]], [s[i], c[i]]])

    rot = rymat(2) @ (rymat(1) @ rymat(0))
    A, B = rot[0, 0], rot[0, 1]
    alpha = abs(B) ** 2
    beta = abs(A) ** 2 - abs(B) ** 2
    gam = A * np.conj(B)
    return np.array([alpha + beta / 2, beta / 2, gam.real, gam.imag],
                    dtype=np.float32)


def kernel(inputs, weights_re, weights_im):
    x = np.asarray(inputs, dtype=np.float32)
    co = _host_coeffs(weights_re, weights_im)
    nc = _get_nc()
    # host-side transpose: [B, 8] -> per core [128 p, 8 q, 128 r]
    xr = x.reshape(N_CORES, P, R, NQ).transpose(0, 1, 3, 2)
    in_maps = [{"x": np.ascontiguousarray(xr[i]), "co": co}
               for i in range(N_CORES)]
    res = run_bass_kernel_spmd(nc, in_maps, list(range(N_CORES)))
    return np.concatenate([res.results[i]["y"] for i in range(N_CORES)])
